# revision 41
# baseline (speedup 1.0000x reference)
"""Trainium2 Bass kernel for nn_BEE_Bin2Symbol (hyper-decoder + masked-conv
autoregressive MLP decoder).

Strategy:
- All 8 cores run identical replicated SPMD work (no collectives; the per-pixel
  recurrence is inherently single-core latency-bound and per-step collectives
  would dominate). Output taken from core 0.
- Phase P (parallel): the two stride-2 5x5 deconvs as phase-decomposed GEMMs
  (weights stationary, [channels, pixels] layout), 3x3 conv, all in float32r
  (1 cyc/row vs 4 for fp32 when N>=256); produces fm1 [384, 1536]. MLP/ctx
  weights are transposed on-device via PE-transpose into [C_in, C_out] layout.
- Sequential phase: 140 slope-3 anti-diagonal wavefronts (t = 3i + j,
  <=16 pixels each — the minimum sequential depth for the type-A 5x5 mask).
  Per step, all GEMMs run activations-stationary / weights-moving so the
  moving dim is the wide one: 12-tap ctx gather (24 MMs, N=384) and the
  6-layer MLP (N=256..512 chunks), each followed by PSUM->SBUF copy (bias
  fused via a 16-row replicated bias tile), PE-transpose back to [C, B],
  and one grouped LeakyReLU on the scalar engine.
- Latency hiding: the 10 "old" ctx taps (age >= 2 steps) for step t+1 are
  computed during step t from a small ring buffer holding the last 9
  wavefronts' outputs (breaks the false interval-overlap dependency on the
  decoded-image writes); only the 2 age-1 taps run on the critical path.
  The residual (w_hat + ep_b5) is pre-added into a padded image at setup.
"""
import sys, os
sys.path.insert(0, "/opt/trn_rl_repo")

import numpy as np

import concourse.bass as bass
import concourse.bacc as bacc
import concourse.mybir as mybir
import concourse.tile as tile
from concourse.masks import make_identity

F32 = mybir.dt.float32
F32R = mybir.dt.float32r  # rounded-fp32 matmul dtype

H, W = 32, 48
HP, WP = H + 4, W + 4            # padded Y image
NPIX = H * W
NSTEPS = 3 * (H - 1) + (W - 1) + 1   # 140

# taps (di, dj): tap pixel = (i-di, j-dj); ctx_w index (ky,kx) = (2-di, 2-dj)
TAPS = [(2, 2), (2, 1), (2, 0), (2, -1), (2, -2),
        (1, 2), (1, 1), (1, 0), (1, -1), (1, -2),
        (0, 1), (0, 2)]

FRESH_TAPS = [(1, -2), (0, 1)]                # age-1 taps (need step t-1)
OLD_TAPS = [d for d in TAPS if d not in FRESH_TAPS]

# MLP layer dims (in, out); L5 out padded 192->256 for fp32r N>=256
LDIMS = [(768, 640), (640, 512), (512, 384), (384, 320), (320, 256), (256, 192)]


def cdiv(a, b):
    return (a + b - 1) // b


def chunks_of(n, c=128):
    """partition chunks [(start, width), ...] of n channels"""
    return [(s, min(c, n - s)) for s in range(0, n, c)]


def _ap(tile_ap, slot_off, elem_off, plist):
    """Build a custom AP into a [128, S, F]-shaped sbuf tile."""
    base = tile_ap[:]
    free = 1
    for d in base.shape[1:]:
        free *= d
    return bass.AP(base.tensor, base.offset + slot_off + elem_off, plist)


def build(nsteps=NSTEPS, mm_dt=F32R):
    nc = bacc.Bacc()

    # ---------------- DRAM I/O ----------------
    di = {}
    di['z_hat'] = nc.dram_tensor('z_hat', [1, 192, 8, 12], F32, kind="ExternalInput")
    di['w_hat'] = nc.dram_tensor('w_hat', [1, 192, 32, 48], F32, kind="ExternalInput")
    di['hs_dw0'] = nc.dram_tensor('hs_dw0', [192, 192, 5, 5], F32, kind="ExternalInput")
    di['hs_db0'] = nc.dram_tensor('hs_db0', [192], F32, kind="ExternalInput")
    di['hs_dw1'] = nc.dram_tensor('hs_dw1', [192, 288, 5, 5], F32, kind="ExternalInput")
    di['hs_db1'] = nc.dram_tensor('hs_db1', [288], F32, kind="ExternalInput")
    di['hs_cw2'] = nc.dram_tensor('hs_cw2', [384, 288, 3, 3], F32, kind="ExternalInput")
    di['hs_cb2'] = nc.dram_tensor('hs_cb2', [384], F32, kind="ExternalInput")
    di['ctx_w'] = nc.dram_tensor('ctx_w', [384, 192, 5, 5], F32, kind="ExternalInput")
    di['ctx_b'] = nc.dram_tensor('ctx_b', [384], F32, kind="ExternalInput")
    for li, (cin, cout_real) in enumerate(LDIMS):
        co = cout_real if li < 5 else 192
        di[f'ep_w{li}'] = nc.dram_tensor(f'ep_w{li}', [co, cin], F32, kind="ExternalInput")
        di[f'ep_b{li}'] = nc.dram_tensor(f'ep_b{li}', [co], F32, kind="ExternalInput")
    out = nc.dram_tensor('out', [1, 192, 32, 48], F32, kind="ExternalOutput")

    with tile.TileContext(nc) as tc:
        with tc.tile_pool(name="persist", bufs=1) as pp, \
             tc.tile_pool(name="mmps", bufs=3, space="PSUM") as mmps, \
             tc.tile_pool(name="ctxps", bufs=2, space="PSUM") as ctxps, \
             tc.tile_pool(name="tps", bufs=3, space="PSUM") as tps:

            ident = pp.tile([128, 128], F32)
            make_identity(nc, ident[:])

            # ---------- persistent state ----------
            Yimg = pp.tile([128, 2, HP * WP], F32R)    # decoded image, padded
            nc.gpsimd.memset(Yimg[:].bitcast(F32), 0.0)
            wimg = pp.tile([128, 2, HP * WP], F32)    # w_hat + b5, padded
            nc.gpsimd.memset(wimg[:], 0.0)
            # ring shadow of the last 9 wavefronts: slot s%9, col = row_i + 2
            Yring = pp.tile([128, 2, 9, 36], F32R)
            nc.gpsimd.memset(Yring[:].bitcast(F32), 0.0)
            fm1 = pp.tile([128, 3, NPIX], F32R)        # conv2 output [384, 1536]

            # transposed weights (persistent)
            WT = []
            for li, (cin, cout) in enumerate(LDIMS):
                kt = cdiv(cin, 128)
                nfree = cout if li < 5 else 256
                w = pp.tile([128, kt, nfree], F32R, tag=f"W{li}T")
                if li == 4:
                    nc.gpsimd.memset(w[64:128, 2, :].bitcast(F32), 0.0)
                elif li == 5:
                    nc.gpsimd.memset(w[:, :, 192:256].bitcast(F32), 0.0)
                WT.append(w)
            WdT = []
            for d in TAPS:
                w = pp.tile([128, 2, 384], F32R, tag=f"Wd{d[0]}_{d[1]}")
                nc.gpsimd.memset(w[64:128, 1, :].bitcast(F32), 0.0)
                WdT.append(w)

            # biases: per-layer [128, kt-chunks] column layout
            def load_bias(name, n):
                nch = cdiv(n, 128)
                t = pp.tile([128, nch], F32, tag=f"b_{name}")
                nc.vector.memset(t[:], 0.0)
                for ci, (s, w_) in enumerate(chunks_of(n)):
                    nc.sync.dma_start(t[0:w_, ci:ci + 1], di[name][s:s + w_][:, None])
                return t
            b_ep = [load_bias(f'ep_b{li}', LDIMS[li][1] if li < 5 else 192) for li in range(6)]
            b_ctx = load_bias('ctx_b', 384)

            # bias tiles replicated to 16 partitions for copy-fused bias adds
            BOFF = {}
            _off = 0
            for li in range(5):
                BOFF[li] = _off; _off += LDIMS[li][1]
            BOFF['ctx'] = _off; _off += 384
            b16 = pp.tile([16, _off], F32)
            ones16 = pp.tile([1, 16], F32)
            nc.vector.memset(ones16[:], 1.0)
            for li in range(5):
                nc.sync.dma_start(b16[0:1, BOFF[li]:BOFF[li] + LDIMS[li][1]],
                                  di[f'ep_b{li}'].ap()[None, :])
            nc.sync.dma_start(b16[0:1, BOFF['ctx']:BOFF['ctx'] + 384],
                              di['ctx_b'].ap()[None, :])
            for c0 in range(0, _off, 512):
                cw_ = min(512, _off - c0)
                bps = mmps.tile([16, 512], F32, tag="mm")
                nc.tensor.matmul(bps[0:16, 0:cw_], ones16[0:1, :],
                                 b16[0:1, c0:c0 + cw_].bitcast(F32),
                                 start=True, stop=True, skip_group_check=True)
                nc.vector.tensor_copy(b16[0:16, c0:c0 + cw_], bps[0:16, 0:cw_])
            b_d0 = load_bias('hs_db0', 192)
            b_d1 = load_bias('hs_db1', 288)
            b_c2 = load_bias('hs_cb2', 384)

            # w_hat into padded image, then += b5
            whv = di['w_hat'].ap()[0]  # [192, 32, 48]
            for ci, (s, cw) in enumerate(chunks_of(192)):
                dst = _ap(wimg, ci * HP * WP, 2 * WP + 2,
                          [[2 * HP * WP, cw], [WP, H], [1, W]])
                nc.sync.dma_start(dst, whv[s:s + cw])
                nc.vector.tensor_tensor(
                    wimg[0:cw, ci, :], wimg[0:cw, ci, :],
                    b_ep[5][0:cw, ci][:, None].to_broadcast((cw, HP * WP)),
                    mybir.AluOpType.add)

            # ================= PHASE P =================
            def deconv_chunk(inp_t, inp_hw, w_t, cin, ms, mw, mi, out_t, bias_t):
                """One out-channel chunk (ms, mw) of a stride-2 k5 deconv.
                w_t: [128, 2, mw*25] weight tile for this chunk."""
                hi, wi = inp_hw
                ip_w = wi + 2
                op_w = 2 * wi + 2
                for py in range(2):
                    for px in range(2):
                        ps = mmps.tile([mw, hi * wi], F32, tag="mm")
                        first = True
                        taps = [(u, v) for u in range(py, 5, 2) for v in range(px, 5, 2)]
                        for ti, (u, v) in enumerate(taps):
                            dy = (py + 2 - u) // 2
                            dx = (px + 2 - v) // 2
                            for ci, (cs, cww) in enumerate(chunks_of(cin)):
                                lhsT = _ap(w_t, ci * 128 * 25, u * 5 + v,
                                           [[2 * 128 * 25, 128], [25, mw]])
                                rhs = _ap(inp_t, ci * (hi + 2) * ip_w,
                                          (1 + dy) * ip_w + (1 + dx),
                                          [[2 * (hi + 2) * ip_w, 128], [ip_w, hi], [1, wi]])
                                last = (ti == len(taps) - 1) and (ci == len(chunks_of(cin)) - 1)
                                nc.tensor.matmul(ps[:], lhsT, rhs,
                                                 start=first, stop=last)
                                first = False
                        dst = _ap(out_t, mi * (2 * hi + 2) * op_w,
                                  (py + 1) * op_w + (px + 1),
                                  [[out_t.shape[1] * (2 * hi + 2) * op_w, mw], [2 * op_w, hi], [2, wi]])
                        nc.scalar.activation(dst, ps[:].rearrange("p (a b) -> p a b", a=hi),
                                             mybir.ActivationFunctionType.Lrelu,
                                             bias=bias_t[0:mw, mi][:, None], alpha=0.01)

            with tc.tile_pool(name="mid", bufs=1) as pmid:
                m1 = pmid.tile([128, 2, 18 * 26], F32R)   # padded 18x26
                nc.gpsimd.memset(m1[:].bitcast(F32), 0.0)
                m2 = pmid.tile([128, 3, 34 * 50], F32R)   # padded 34x50
                nc.gpsimd.memset(m2[:].bitcast(F32), 0.0)

                # ---- deconv0: z[192,8,12] -> m1[192,16,24] ----
                with tc.tile_pool(name="st1", bufs=1) as pst, \
                     tc.tile_pool(name="st1b", bufs=2) as pstb:
                    zps = pst.tile([128, 2, 10 * 14], F32)
                    nc.gpsimd.memset(zps[:], 0.0)
                    zv = di['z_hat'].ap()[0]
                    for ci, (s, cw) in enumerate(chunks_of(192)):
                        dst = _ap(zps, ci * 140, 14 + 1, [[2 * 140, cw], [14, 8], [1, 12]])
                        nc.sync.dma_start(dst, zv[s:s + cw])
                    zp = pst.tile([128, 2, 10 * 14], F32R)
                    nc.vector.tensor_copy(zp[:], zps[:])
                    for mi, (ms, mw) in enumerate(chunks_of(192)):
                        dw = pst.tile([128, 2, 128 * 25], F32R, tag="dwc")
                        nc.gpsimd.memset(dw[64:128, 1, :].bitcast(F32), 0.0)
                        for ci, (cs, cww) in enumerate(chunks_of(192)):
                            for hh in range(3):
                                o0, o1 = hh * mw // 3, (hh + 1) * mw // 3
                                if o0 == o1:
                                    continue
                                dws = pstb.tile([128, 43 * 25], F32, tag="dwcs")
                                nc.sync.dma_start(
                                    dws[0:cww, 0:(o1 - o0) * 25],
                                    di['hs_dw0'].ap()[cs:cs + cww, ms + o0:ms + o1].rearrange("c o kh kw -> c (o kh kw)"))
                                nc.vector.tensor_copy(dw[0:cww, ci, o0 * 25:o1 * 25], dws[0:cww, 0:(o1 - o0) * 25])
                        deconv_chunk(zp, (8, 12), dw, 192, ms, mw, mi, m1, b_d0)

                # ---- deconv1: m1[192,16,24] -> m2[288,32,48] ----
                with tc.tile_pool(name="st2", bufs=1) as pst, \
                     tc.tile_pool(name="st2b", bufs=2) as pstb:
                    for mi, (ms, mw) in enumerate(chunks_of(288)):
                        dw = pst.tile([128, 2, 128 * 25], F32R, tag="dwc")
                        nc.gpsimd.memset(dw[64:128, 1, :].bitcast(F32), 0.0)
                        for ci, (cs, cww) in enumerate(chunks_of(192)):
                            for hh in range(3):
                                o0, o1 = hh * mw // 3, (hh + 1) * mw // 3
                                if o0 == o1:
                                    continue
                                dws = pstb.tile([128, 43 * 25], F32, tag="dwcs")
                                nc.sync.dma_start(
                                    dws[0:cww, 0:(o1 - o0) * 25],
                                    di['hs_dw1'].ap()[cs:cs + cww, ms + o0:ms + o1].rearrange("c o kh kw -> c (o kh kw)"))
                                nc.vector.tensor_copy(dw[0:cww, ci, o0 * 25:o1 * 25], dws[0:cww, 0:(o1 - o0) * 25])
                        deconv_chunk(m1, (16, 24), dw, 192, ms, mw, mi, m2, b_d1)

                # ---- conv2 3x3: m2[288,32,48] -> fm1[384,1536], by out thirds ----
                with tc.tile_pool(name="st3", bufs=1) as pst, \
                     tc.tile_pool(name="st3b", bufs=2) as pstb:
                    for mi in range(3):
                        cw2 = pstb.tile([128, 288 * 9], F32, tag="cw2")
                        nc.sync.dma_start(
                            cw2[:],
                            di['hs_cw2'].ap()[mi * 128:(mi + 1) * 128].rearrange("o c kh kw -> o (c kh kw)"))
                        cw2T = pst.tile([128, 3, 9 * 128], F32R, tag="cw2T")
                        nc.gpsimd.memset(cw2T[:, 2, :].bitcast(F32), 0.0)
                        for k in range(9):
                            for si, (ss, sw) in enumerate(chunks_of(288)):
                                src = _ap(cw2, 0, ss * 9 + k, [[288 * 9, 128], [9, sw]])
                                pt = tps.tile([128, 128], F32, tag="tp")
                                nc.tensor.transpose(pt[0:sw, 0:128], src, ident[:])
                                nc.vector.tensor_copy(cw2T[0:sw, si, k * 128:(k + 1) * 128],
                                                      pt[0:sw, 0:128])
                        for ch in range(4):
                            ps = mmps.tile([128, 384], F32, tag="mm")
                            first = True
                            for k in range(9):
                                ky, kx = k // 3, k % 3
                                for si, (ss, sw) in enumerate(chunks_of(288)):
                                    lhsT = cw2T[:, si, k * 128:(k + 1) * 128]
                                    rhs = _ap(m2, si * 34 * 50, (ky + 8 * ch) * 50 + kx,
                                              [[3 * 34 * 50, 128], [50, 8], [1, 48]])
                                    last = (k == 8) and (si == 2)
                                    nc.tensor.matmul(ps[:], lhsT, rhs,
                                                     start=first, stop=last)
                                    first = False
                            nc.scalar.activation(fm1[:, mi, ch * 384:(ch + 1) * 384], ps[:],
                                                 mybir.ActivationFunctionType.Identity,
                                                 bias=b_c2[:, mi][:, None], alpha=0.0)

            # ---- MLP weight transposes ----
            with tc.tile_pool(name="st4", bufs=2) as pst:
                def load_and_transpose(dram, n_out, n_in, dstT):
                    wnat = pst.tile([128, 6, 768], F32, tag="wnat")
                    for mi, (ms, mw) in enumerate(chunks_of(n_out)):
                        nc.sync.dma_start(wnat[0:mw, mi, 0:n_in], dram[ms:ms + mw])
                    for ci, (cs, cww) in enumerate(chunks_of(n_in)):
                        for mi, (ms, mw) in enumerate(chunks_of(n_out)):
                            pt = tps.tile([128, 128], F32, tag="tp")
                            nc.tensor.transpose(pt[0:cww, 0:mw], wnat[0:mw, mi, cs:cs + cww], ident[0:mw, 0:mw])
                            nc.vector.tensor_copy(dstT[0:cww, ci, ms:ms + mw], pt[0:cww, 0:mw])

                for li, (cin, cout) in enumerate(LDIMS):
                    co_real = cout if li < 5 else 192
                    load_and_transpose(di[f'ep_w{li}'].ap(), co_real, cin, WT[li])

            # ctx taps, by out thirds
            with tc.tile_pool(name="st5", bufs=2) as pst:
                for mi in range(3):
                    cwn = pst.tile([128, 192 * 25], F32, tag="cwn")
                    nc.sync.dma_start(
                        cwn[:],
                        di['ctx_w'].ap()[mi * 128:(mi + 1) * 128].rearrange("o c kh kw -> o (c kh kw)"))
                    for ti, (dy, dx) in enumerate(TAPS):
                        ky, kx = 2 - dy, 2 - dx
                        for ci, (cs, cww) in enumerate(chunks_of(192)):
                            src = _ap(cwn, 0, cs * 25 + ky * 5 + kx, [[192 * 25, 128], [25, cww]])
                            pt = tps.tile([128, 128], F32, tag="tp")
                            nc.tensor.transpose(pt[0:cww, 0:128], src, ident[:])
                            nc.vector.tensor_copy(WdT[ti][0:cww, ci, mi * 128:(mi + 1) * 128],
                                                  pt[0:cww, 0:128])

            # ================= SEQUENTIAL PHASE =================
            # X tiles (persistent, zero-init so sparse rows stay 0)
            X = []
            for li, (cin, cout) in enumerate(LDIMS):
                x = pp.tile([128, 3 if li == 0 else cdiv(cin, 128), 16], F32R, tag=f"X{li}")
                if li == 4:
                    nc.gpsimd.memset(x[64:128, 2, :].bitcast(F32), 0.0)
                X.append(x)

            def ydiag(src, slot, i0, j0, B, step=49):
                """[128, B] diagonal AP into padded image tile (Yimg/wimg)"""
                off = (i0 + 2) * WP + (j0 + 2)
                return _ap(src, slot * HP * WP, off, [[2 * HP * WP, 128], [step, B]])

            def step_geom(t):
                i_lo = max(0, cdiv(t - (W - 1), 3))
                i_hi = min(H - 1, t // 3)
                return i_lo, i_hi - i_lo + 1, t - 3 * i_lo

            def emit_ctx_mms(cps, t, taps, start):
                """Accumulate tap GEMMs for step t into psum cps (reads ring).
                Skips taps whose source wavefront is < 0 (zero border)."""
                i_lo, B, j_lo = step_geom(t)
                for (dy, dx) in taps:
                    ti = TAPS.index((dy, dx))
                    s = t - (3 * dy + dx)
                    if s < 0:
                        continue
                    for ci in range(2):
                        col0 = i_lo - dy + 2
                        lhsT = _ap(Yring, ci * 9 * 36 + (s % 9) * 36, col0,
                                   [[2 * 9 * 36, 128], [1, B]])
                        nc.tensor.matmul(cps[0:B, :], lhsT, WdT[ti][:, ci, :],
                                         start=start, stop=False,
                                         skip_group_check=True)
                        start = False
                return start

            def finish_ctx(cps, t, start=False):
                """Fresh taps (stop on last)."""
                i_lo, B, j_lo = step_geom(t)
                for k, (dy, dx) in enumerate(FRESH_TAPS):
                    ti = TAPS.index((dy, dx))
                    for ci in range(2):
                        lhsT = ydiag(Yimg, ci, i_lo - dy, j_lo + dx, B)
                        nc.tensor.matmul(cps[0:B, :], lhsT, WdT[ti][:, ci, :],
                                         start=start,
                                         stop=(k == 1 and ci == 1),
                                         skip_group_check=True)
                        start = False

            # prologue: step-0 ctx (no old sources exist; fresh MMs open group)
            cps_cur = ctxps.tile([16, 384], F32, tag="ctx")
            cur_start = True

            OLD_BATCHES = [OLD_TAPS[0:2], OLD_TAPS[2:4], OLD_TAPS[4:6], OLD_TAPS[6:8], OLD_TAPS[8:10], []]

            for t in range(nsteps):
                i_lo, B, j_lo = step_geom(t)
                if t >= 1:
                    p_lo, pB, p_jlo = step_geom(t - 1)
                    sl9 = (t - 1) % 9
                    nc.vector.memset(Yring[:, :, sl9, :].bitcast(F32), 0.0)
                    for c, (cs, cw) in enumerate(chunks_of(192)):
                        sY = ydiag(Yimg, c, p_lo, p_jlo, pB)
                        dY = _ap(Yring, c * 9 * 36 + sl9 * 36, p_lo + 2,
                                 [[2 * 9 * 36, cw], [1, pB]])
                        nc.vector.tensor_copy(
                            dY, bass.AP(sY.tensor, sY.offset, [[2 * HP * WP, cw], [49, pB]]))
                finish_ctx(cps_cur, t, start=cur_start)
                cur_start = True

                cps_next = None
                if t + 1 < nsteps:
                    cps_next = ctxps.tile([16, 384], F32, tag="ctx")
                nxt_start = True

                # L0 f-part MMs (independent of ctx) overlap the ctx consume
                l0ps = [mmps.tile([16, 320], F32, tag="mm") for _ in range(2)]
                for ch in range(2):
                    for k in range(3):
                        lhsT = _ap(fm1, k * NPIX, i_lo * W + j_lo, [[3 * NPIX, 128], [W - 3, B]])
                        nc.tensor.matmul(l0ps[ch][0:B, :], lhsT,
                                         WT[0][:, k, ch * 320:(ch + 1) * 320],
                                         start=(k == 0), stop=False,
                                         skip_group_check=True)

                if cps_next is not None:
                    nxt_start = emit_ctx_mms(cps_next, t + 1, OLD_BATCHES[0], nxt_start)
                    cur_start = nxt_start

                # consume ctx: bias-fused copy -> grouped transpose -> X0 copy
                sc = pp.tile([16, 640], F32, tag="s_ctx")
                nc.vector.tensor_tensor(sc[0:B, 0:384], cps_cur[0:B, 0:384],
                                        b16[0:B, BOFF['ctx']:BOFF['ctx'] + 384],
                                        mybir.AluOpType.add)
                ptg = tps.tile([128, 8, 16], F32, tag="tp")
                for c in range(3):
                    nc.tensor.transpose(ptg[:, c, 0:B], sc[0:B, c * 128:(c + 1) * 128], ident[0:B, 0:B])
                nc.vector.tensor_copy(X[0][:, :, 0:B], ptg[:, 0:3, 0:B])

                # ---- MLP ----
                for li, (cin, cout) in enumerate(LDIMS):
                    nfree = cout if li < 5 else 256
                    kt = cdiv(cin, 128)
                    nchunks = 2 if li <= 1 else 1
                    csz = nfree // nchunks
                    sl = pp.tile([16, 640], F32, tag=f"s_l{li % 2}")
                    for ch in range(nchunks):
                        if li == 0:
                            ps = l0ps[ch]
                            for k in range(3):
                                nc.tensor.matmul(
                                    ps[0:B, :], X[0][:, k, 0:B],
                                    WT[0][:, 3 + k, ch * csz:(ch + 1) * csz],
                                    start=False, stop=(k == 2),
                                    skip_group_check=True)
                        else:
                            ps = mmps.tile([16, csz], F32, tag="mm")
                            for k in range(kt):
                                nc.tensor.matmul(
                                    ps[0:B, :], X[li][:, k, 0:B],
                                    WT[li][:, k, ch * csz:(ch + 1) * csz],
                                    start=(k == 0), stop=(k == kt - 1),
                                    skip_group_check=True)
                        cpw = csz if li < 5 else 192
                        if li < 5:
                            nc.vector.tensor_tensor(
                                sl[0:B, ch * csz:ch * csz + cpw], ps[0:B, 0:cpw],
                                b16[0:B, BOFF[li] + ch * csz:BOFF[li] + ch * csz + cpw],
                                mybir.AluOpType.add)
                        else:
                            nc.vector.tensor_copy(sl[0:B, 0:cpw], ps[0:B, 0:cpw])
                    # fill PE gap with next step's old-ctx tap GEMMs
                    if cps_next is not None and li < 5:
                        nxt_start = emit_ctx_mms(cps_next, t + 1, OLD_BATCHES[li + 1], nxt_start)
                        cur_start = nxt_start
                    co_real = cout if li < 5 else 192
                    nch = cdiv(co_real, 128)
                    ptg = tps.tile([128, 8, 16], F32, tag="tp")
                    for c, (cs, cw) in enumerate(chunks_of(co_real)):
                        nc.tensor.transpose(ptg[0:cw, c, 0:B], sl[0:B, cs:cs + cw], ident[0:B, 0:B])
                    if li < 5:
                        if co_real % 128 == 0:
                            nc.scalar.activation(X[li + 1][:, :, 0:B], ptg[:, 0:nch, 0:B],
                                                 mybir.ActivationFunctionType.Lrelu, alpha=0.01)
                        else:
                            lw = co_real % 128
                            nc.scalar.activation(X[li + 1][:, 0:nch - 1, 0:B], ptg[:, 0:nch - 1, 0:B],
                                                 mybir.ActivationFunctionType.Lrelu, alpha=0.01)
                            nc.scalar.activation(X[li + 1][0:lw, nch - 1, 0:B], ptg[0:lw, nch - 1, 0:B],
                                                 mybir.ActivationFunctionType.Lrelu, alpha=0.01)
                    else:
                        for c, (cs, cw) in enumerate(chunks_of(co_real)):
                            dstY = ydiag(Yimg, c, i_lo, j_lo, B)
                            srcW = ydiag(wimg, c, i_lo, j_lo, B)
                            nc.vector.tensor_tensor(
                                bass.AP(dstY.tensor, dstY.offset, [[2 * HP * WP, cw], [49, B]]),
                                ptg[0:cw, c, 0:B],
                                bass.AP(srcW.tensor, srcW.offset, [[2 * HP * WP, cw], [49, B]]),
                                mybir.AluOpType.add)
                cps_cur = cps_next

            # ---- output DMA ----
            ov = out.ap()[0]  # [192, 32, 48]
            for ci, (s, cw) in enumerate(chunks_of(192)):
                src = _ap(Yimg, ci * HP * WP, 2 * WP + 2,
                          [[2 * HP * WP, cw], [WP, H], [1, W]])
                nc.sync.dma_start(ov[s:s + cw], src.bitcast(F32))

    nc.compile()
    return nc


_NC_CACHE = {}


def kernel(**inputs):
    from concourse.bass_utils import run_bass_kernel_spmd
    key = "full"
    if key not in _NC_CACHE:
        _NC_CACHE[key] = build()
    nc = _NC_CACHE[key]
    in_map = {k: np.ascontiguousarray(np.asarray(v, dtype=np.float32)) for k, v in inputs.items()}
    res = run_bass_kernel_spmd(nc, [in_map] * 8, core_ids=list(range(8)))
    return res.results[0]['out']


if __name__ == "__main__":
    t = build(nsteps=int(sys.argv[1]) if len(sys.argv) > 1 else NSTEPS)
    print("build ok")
